# revision 37
# baseline (speedup 1.0000x reference)
"""Multi-Head Latent Attention (MLA) Trainium2 kernel — v2 (bf16).

Problem (hardcoded): B=2, S=2048, D_MODEL=2048, H=16, HEAD_DIM=128,
D_LATENT=512 (D_QK=256 / D_V=256), ROPE_DIM=64, fp32 in/out.

Reference semantics: q = concat([q_no_rope(1024), q_rope(1024)]).reshape(16
heads x 128), so heads 0-7 take both 64-dim halves from the latent
decompression and heads 8-15 take both halves from the rope projection of x;
RoPE rotates dims 64:128 of every head.

Sharding: 8 cores = 2 batches x 4 head-groups; core (b, hg) owns heads
[2hg, 2hg+1, 8+2hg, 8+2hg+1] (2 decompression + 2 rope-projection heads),
computes the shared latent for its batch redundantly, and produces a partial
output projection (its heads' rows of W_out), transposed [e, q]. The host
sums the 4 partials per batch (in f32; device emits bf16 partials).

v2 changes vs v1:
  - all matmul operands bf16 (x, weights, latents, q/k, exp, v, ctx):
    halves HBM+SBUF traffic, bf16 matmuls measure faster than fp32r,
    and bf16 allows N=1024 moving operands (halved instruction count in
    stage 1 / phase 2 dec).
  - stage 1 is a single pass over xT (w_big fully SBUF-resident in bf16),
    removing the second 16MB xT read that starved DMA in v1.
  - softmax denominator: 15/16 of the ones-matmuls replaced by a DVE
    f32 add-tree over the 16 exp tiles; one PE colsum matmul per
    (q-chunk, head) remains.
  - output partials staged+DMA'd in bf16 (halves the tail drain).

On-chip layout is feature-major so matmuls contract over partitions.
Pipeline:
  stage1:  bigT = w_big^T @ xT, single pass, N=1024 chunks
           (m 0:4 -> latT tiles, m 4:8 -> rope-head raw qkT rows)
  phase2:  qkT[dec][0:128] = W_{q,k}_dec^T @ c_qkT  (N=1024)
           v_nat[S,512]    = c_v @ W_v slice        (natural layout)
  rope:    qkT[i][64:128] = raw*cos4 + swap32(raw)*sin4s  (in place)
  attn:    per (q-chunk 512, head): scoresT[k,q] psum = kT^T @ qT (K=128);
           expT = exp(scoresT/sqrt(128)) bf16; ctxT += v-block^T @ expT;
           acc = DVE add-tree(expT tiles) f32; den psum = ones^T @ acc
           (all psum rows = column sums -> free broadcast);
           ctxT *= 1/den (reciprocal_approx_fast)
  out:     outT[e,q] = W_out_part^T @ ctxT, fused per q-chunk, bf16 out
"""

import math

import numpy as np

B = 2
S = 2048
D = 2048
H4 = 4            # heads per core
HD = 128          # head dim
DL = 512          # d_latent
DQK = 256
RD = 64           # rope dim
NC = 8            # cores

SCALE = 1.0 / math.sqrt(HD)

_prog_cache = {}


def _build_program(phases=4):
    import concourse.tile as tile
    from concourse import bacc, mybir

    bf16 = mybir.dt.bfloat16
    f32 = mybir.dt.float32
    f32r = mybir.dt.float32r

    nc = bacc.Bacc("TRN2", target_bir_lowering=False, debug=False, num_devices=1)

    xT = nc.dram_tensor("xT", [D, S], bf16, kind="ExternalInput")
    w_big = nc.dram_tensor("w_big", [D, 1024], bf16, kind="ExternalInput")
    w_qk = nc.dram_tensor("w_qk", [DQK, 512], bf16, kind="ExternalInput")
    w_v = nc.dram_tensor("w_v", [DQK, 512], bf16, kind="ExternalInput")
    w_o = nc.dram_tensor("w_o", [DL, D], bf16, kind="ExternalInput")
    cos4_d = nc.dram_tensor("cos4", [128, S], bf16, kind="ExternalInput")
    sin4s_d = nc.dram_tensor("sin4s", [64, S], bf16, kind="ExternalInput")
    out_d = nc.dram_tensor("out", [D, S], bf16, kind="ExternalOutput")

    NQ = S // 512    # 4 q chunks of 512 (attention)
    NK = S // 128    # 16 k/seq chunks of 128
    KD = D // 128    # 16 contraction chunks for stage 1
    N4 = S // 1024   # 2 wide n-chunks of 1024 (stage1/phase2)

    with tile.TileContext(nc, pool_alloc_mode="queue") as tc:
        import contextlib

        with contextlib.ExitStack() as ctx:
            # persistent pools (live to end of program, LIFO via ExitStack)
            ones_p = ctx.enter_context(tc.tile_pool(name="onesp", bufs=1))
            qk_p = ctx.enter_context(tc.tile_pool(name="qk", bufs=1))
            v_p = ctx.enter_context(tc.tile_pool(name="vp", bufs=1))

            ones_f32 = ones_p.tile([128, 128], f32)
            nc.gpsimd.memset(ones_f32[:], 1.0)
            ones_r = ones_p.tile([128, 128], f32r)
            nc.vector.tensor_copy(ones_r[:], ones_f32[:])
            warm = ones_p.tile([128, 1], f32)
            nc.scalar.activation(warm[:], ones_f32[:, 0:1],
                                 mybir.ActivationFunctionType.Exp)
            # per-head assembled q/k: rows 0:64 nr dims, 64:128 roped dims
            # qkT[0..3] = q heads 0..3, qkT[4..7] = k heads 0..3
            qkT = [qk_p.tile([128, S], bf16, name=f"qkT{i}", tag=f"qk{i}")
                   for i in range(8)]
            v_nat = [v_p.tile([128, 512], bf16, name=f"v{i}", tag=f"v{i}")
                     for i in range(NK)]

            consts_cm = tc.tile_pool(name="consts", bufs=1)
            consts = consts_cm.__enter__()
            swp_cm = tc.tile_pool(name="swpA", bufs=2)
            swp_p = swp_cm.__enter__()
            scr_cm = tc.tile_pool(name="scrA", bufs=2)
            scr_p = scr_cm.__enter__()

            # ---------------- stage 1: bigT = w_big^T @ xT -----------------
            # single pass; w_big fully resident (bf16), xT read once as
            # [128,1024] tiles; per 512-wide n-chunk all 8 m-tiles
            # accumulate in 8 psum banks (matmul out is 1 bank: N<=512).
            ps1_cm = tc.tile_pool(name="ps1", bufs=8, space="PSUM")
            ps1_p = ps1_cm.__enter__()

            wbig_cm = tc.tile_pool(name="wbig", bufs=1)
            wbig_p = wbig_cm.__enter__()
            xt_cm = tc.tile_pool(name="xt", bufs=18)
            xt_p = xt_cm.__enter__()
            wdec_cm = tc.tile_pool(name="wdec", bufs=1)
            wdec_p = wdec_cm.__enter__()
            lat_cm = tc.tile_pool(name="lat", bufs=1)
            lat_p = lat_cm.__enter__()
            # latn[l][n4]: latent rows l*128:(l+1)*128, cols n4*1024:+1024
            latn = [[lat_p.tile([128, 1024], bf16, name=f"latT{i}_{n}",
                                tag=f"lat{i}_{n}") for n in range(N4)]
                    for i in range(4)]

            wbig_sb = [wbig_p.tile([128, 1024], bf16, name=f"wb{k}",
                                   tag=f"wb{k}") for k in range(KD)]
            for n4 in range(N4):
                xts = []
                for k in range(KD):
                    if n4 == 0:
                        nc.sync.dma_start(wbig_sb[k][:],
                                          w_big.ap()[k * 128:(k + 1) * 128, :])
                    x_t = xt_p.tile([128, 1024], bf16, name="xt", tag="xt")
                    nc.sync.dma_start(
                        x_t[:], xT.ap()[k * 128:(k + 1) * 128,
                                        n4 * 1024:(n4 + 1) * 1024])
                    xts.append(x_t)
                if n4 == 0:
                    # phase-2 + attention weights: queue after the first
                    # stage-1 chunk's loads, well before their first use
                    wqk_sb = []
                    for l in range(2):
                        w_t = wdec_p.tile([128, 512], bf16, name=f"wqk{l}",
                                          tag=f"wqk{l}")
                        nc.sync.dma_start(w_t[:],
                                          w_qk.ap()[l * 128:(l + 1) * 128, :])
                        wqk_sb.append(w_t)
                    wv_sb = []
                    for l in range(2):
                        w_t = wdec_p.tile([128, 512], bf16, name=f"wv{l}",
                                          tag=f"wv{l}")
                        nc.sync.dma_start(w_t[:],
                                          w_v.ap()[l * 128:(l + 1) * 128, :])
                        wv_sb.append(w_t)
                    cos4 = consts.tile([128, S], bf16)
                    nc.sync.dma_start(cos4[:], cos4_d.ap()[:])
                    sin4s = consts.tile([64, S], bf16)
                    nc.sync.dma_start(sin4s[:], sin4s_d.ap()[:])
                for sub in range(2):
                    psums = [ps1_p.tile([128, 512], f32, name=f"ps1_{m}",
                                        tag="ps1") for m in range(8)]
                    for k in range(KD):
                        for m in range(8):
                            nc.tensor.matmul(
                                psums[m][:],
                                wbig_sb[k][:, m * 128:(m + 1) * 128],
                                xts[k][:, sub * 512:(sub + 1) * 512],
                                start=(k == 0),
                                stop=(k == KD - 1),
                            )
                    lsl = slice(sub * 512, (sub + 1) * 512)
                    nsl = slice(n4 * 1024 + sub * 512,
                                n4 * 1024 + (sub + 1) * 512)
                    for m in range(8):
                        if m < 4:
                            nc.vector.tensor_copy(latn[m][n4][:, lsl],
                                                  psums[m][:])
                        else:
                            dst = qkT[[2, 3, 6, 7][m - 4]]
                            nc.vector.tensor_copy(dst[:, nsl], psums[m][:])

            if phases == 1:
                for i in range(4):
                    for n in range(N4):
                        nc.sync.dma_start(
                            out_d.ap()[i * 128:(i + 1) * 128,
                                       n * 1024:(n + 1) * 1024],
                            latn[i][n][:])
                for i, t in enumerate(qkT):
                    nc.sync.dma_start(
                        out_d.ap()[512 + i * 128:512 + (i + 1) * 128, :], t[:])

            # ---------------- rope helper (in place, qkT rows 64:128) ------
            # all scratch in rows 0:64 (cos/sin pattern repeats every 32
            # rows, so base-0 slices align); only the final add writes the
            # qkT rope rows at base 64. `eng` picks the engine: DVE for the
            # latency-critical tiles, GpSimd for the ones with slack.
            def rope_tiles(swp_p, scr_p, idxs, eng):
                for i in idxs:
                    t = qkT[i]
                    sw = swp_p.tile([64, S], bf16, name=f"sw{i}", tag="sw")
                    nc.sync.dma_start(sw[0:32, :], t[96:128, :])
                    nc.sync.dma_start(sw[32:64, :], t[64:96, :])
                    tmp_sin = scr_p.tile([64, S], bf16, name="tsin", tag="scr")
                    eng.tensor_mul(tmp_sin[0:64, :], sw[0:64, :],
                                   sin4s[0:64, :])
                    tmp_cos = scr_p.tile([64, S], bf16, name="tcos", tag="scr")
                    eng.tensor_mul(tmp_cos[0:64, :], t[64:128, :],
                                   cos4[64:128, :])
                    eng.tensor_add(t[64:128, :], tmp_cos[0:64, :],
                                   tmp_sin[0:64, :])

            if phases >= 2:
                # heads 2,6 (x-projection) only need stage-1 output; rope
                # them on DVE right away so attention h=2 starts right
                # after phase 2's PE work.
                rope_tiles(swp_p, scr_p, [2, 6], nc.vector)

            # ---------------- phase 2: nr decompression + v ----------------
            # v-loop first (AV deps), v copies on DVE, dec copies on ACT
            # (they finish before the first exp needs the ACT queue); dec
            # mt order 0,2 (q0/k0) then 1,3 so rope(0,4) can start early.
            def phase2_dec(mts):
                for mt in mts:
                    for n in range(NQ):
                        nsl = slice(n * 512, (n + 1) * 512)
                        ps = ps1_p.tile([128, 512], f32, name="ps2",
                                        tag="ps1")
                        for l in range(2):
                            nc.tensor.matmul(
                                ps[:],
                                wqk_sb[l][:, mt * 128:(mt + 1) * 128],
                                latn[l][n // 2][:, (n % 2) * 512:
                                                (n % 2 + 1) * 512],
                                start=(l == 0), stop=(l == 1),
                            )
                        nc.scalar.copy(qkT[[0, 1, 4, 5][mt]][:, nsl], ps[:])

            if phases >= 2:
                for sc in range(NK):
                    ps = ps1_p.tile([128, 512], f32, name="ps2v", tag="ps1")
                    for l in range(2):
                        nc.tensor.matmul(
                            ps[:],
                            latn[2 + l][sc // 8][:, (sc % 8) * 128:
                                                 (sc % 8 + 1) * 128],
                            wv_sb[l][:],
                            start=(l == 0), stop=(l == 1),
                        )
                    if sc % 2 == 0:
                        nc.vector.tensor_copy(v_nat[sc][:], ps[:])
                    else:
                        nc.scalar.copy(v_nat[sc][:], ps[:])
                phase2_dec([0, 2, 1, 3])
            lat_cm.__exit__(None, None, None)
            wdec_cm.__exit__(None, None, None)
            xt_cm.__exit__(None, None, None)
            wbig_cm.__exit__(None, None, None)
            ps1_cm.__exit__(None, None, None)

            if phases >= 2:
                # h=3 pair on DVE right behind the phase-2 v copies; the
                # dec-head pairs (0,4 / 1,5) are emitted interleaved with
                # the attention blocks below (their heads run last).
                rope_tiles(swp_p, scr_p, [3, 7], nc.vector)
                if phases < 4:
                    rope_tiles(swp_p, scr_p, [0, 4, 1, 5], nc.vector)
            if phases == 2:
                for i, t in enumerate(qkT):
                    nc.sync.dma_start(out_d.ap()[i * 128:(i + 1) * 128, :],
                                      t[:])
            if phases == 3:
                for i, t in enumerate(qkT):
                    nc.sync.dma_start(out_d.ap()[i * 128:(i + 1) * 128, :],
                                      t[:])
                for sc in range(NK):
                    nc.sync.dma_start(
                        out_d.ap()[1024 + (sc // 4) * 128:
                                   1024 + (sc // 4 + 1) * 128,
                                   (sc % 4) * 512:(sc % 4 + 1) * 512],
                        v_nat[sc][:])

            # ---------------- phase 3+4: attention + output projection ----
            if phases >= 4:
              with tc.tile_pool(name="wo", bufs=1) as wo_p, \
                 tc.tile_pool(name="exp", bufs=20) as exp_p, \
                 tc.tile_pool(name="den1", bufs=10) as den1_p, \
                 tc.tile_pool(name="den2", bufs=5) as den2_p, \
                 tc.tile_pool(name="den3", bufs=3) as den3_p, \
                 tc.tile_pool(name="acc", bufs=2) as acc_p, \
                 tc.tile_pool(name="ctx", bufs=8) as ctx_p, \
                 tc.tile_pool(name="rden", bufs=2) as rden_p, \
                 tc.tile_pool(name="stage", bufs=4) as stage_p, \
                 tc.tile_pool(name="ps_s", bufs=3, space="PSUM") as ps_s_p, \
                 tc.tile_pool(name="ps_c", bufs=2, space="PSUM") as ps_c_p, \
                 tc.tile_pool(name="ps_d", bufs=1, space="PSUM") as ps_d_p, \
                 tc.tile_pool(name="ps_o", bufs=2, space="PSUM") as ps_o_p:
                wo_sb = []
                for kk in range(4):
                    w_t = wo_p.tile([128, D], bf16, name=f"wo{kk}",
                                    tag=f"wo{kk}")
                    nc.sync.dma_start(w_t[:], w_o.ap()[kk * 128:(kk + 1) * 128, :])
                    wo_sb.append(w_t)

                def emit_group(qc, h):
                    # one (q-chunk, head) attention block
                    qsl = slice(qc * 512, (qc + 1) * 512)
                    ps_ctx = ps_c_p.tile([128, 512], f32, name="psc",
                                         tag="psc")
                    exps = []
                    dlvl1 = []
                    dlvl2 = []

                    # software-pipelined: scores(kc+1) is issued before
                    # av(kc) so PE isn't FIFO-blocked on exp(kc); the DVE
                    # add-tree (softmax denominator) accumulates exp tiles
                    # as they land.
                    def emit_scores(kc):
                        ps_s = ps_s_p.tile([128, 512], f32, name="pss",
                                           tag="pss")
                        nc.tensor.matmul(
                            ps_s[:],
                            qkT[4 + h][:, kc * 128:(kc + 1) * 128],
                            qkT[h][:, qsl],
                            start=True, stop=True,
                        )
                        expT = exp_p.tile([128, 512], bf16, name="expT",
                                          tag="exp")
                        nc.scalar.activation(
                            expT[:], ps_s[:],
                            mybir.ActivationFunctionType.Exp, scale=SCALE)
                        exps.append(expT)
                        if kc % 2 == 1:
                            # lvl1 in bf16: 2x DVE rate; the bf16 rounding
                            # averages out over the 8 partial sums
                            d = den1_p.tile([128, 512], bf16, name="d1",
                                            tag="d1")
                            nc.vector.tensor_add(d[:], exps[kc - 1][:],
                                                 exps[kc][:])
                            dlvl1.append(d)
                            if kc % 4 == 3:
                                j2 = kc // 4
                                d2 = den2_p.tile([128, 512], f32,
                                                 name="d2", tag="d2")
                                nc.vector.tensor_add(
                                    d2[:], dlvl1[j2 * 2][:],
                                    dlvl1[j2 * 2 + 1][:])
                                dlvl2.append(d2)

                    def emit_av(kc):
                        nc.tensor.matmul(
                            ps_ctx[:],
                            v_nat[kc][:, h * 128:(h + 1) * 128],
                            exps[kc][:],
                            start=(kc == 0), stop=(kc == NK - 1),
                        )

                    emit_scores(0)
                    for kc in range(1, NK):
                        emit_scores(kc)
                        emit_av(kc - 1)
                    emit_av(NK - 1)
                    d3a = den3_p.tile([128, 512], f32, name="d3a", tag="d3")
                    nc.vector.tensor_add(d3a[:], dlvl2[0][:], dlvl2[1][:])
                    d3b = den3_p.tile([128, 512], f32, name="d3b", tag="d3")
                    nc.vector.tensor_add(d3b[:], dlvl2[2][:], dlvl2[3][:])
                    acc = acc_p.tile([128, 512], f32r, name="acc", tag="acc")
                    nc.vector.tensor_add(acc[:], d3a[:], d3b[:])
                    ps_den = ps_d_p.tile([128, 512], f32, name="psd",
                                         tag="psd")
                    nc.tensor.matmul(ps_den[:], ones_r[:], acc[:],
                                     start=True, stop=True)
                    rden = rden_p.tile([128, 512], f32, name="rden",
                                       tag="rden")
                    nc.vector.reciprocal_approx_fast(rden[:], ps_den[:])
                    c_t = ctx_p.tile([128, 512], bf16, name="ctxt",
                                     tag="ctx")
                    nc.vector.tensor_mul(c_t[:], ps_ctx[:], rden[:])
                    if phases == 5:
                        r0 = (qc * 4 + h) * 128
                        nc.sync.dma_start(out_d.ap()[r0:r0 + 128, 0:512],
                                          c_t[:])
                    return c_t

                def emit_outproj(qc, ctx_by_head, ms):
                    qsl = slice(qc * 512, (qc + 1) * 512)
                    for m in (ms if phases >= 6 or phases == 4 else []):
                        ps_o = ps_o_p.tile([128, 512], f32, name="pso",
                                           tag="pso")
                        for kk in range(4):
                            nc.tensor.matmul(
                                ps_o[:],
                                wo_sb[kk][:, m * 128:(m + 1) * 128],
                                ctx_by_head[kk][:],
                                start=(kk == 0), stop=(kk == 3),
                            )
                        st = stage_p.tile([128, 512], bf16, name="stg",
                                          tag="stage")
                        if m % 2 == 0:
                            nc.vector.tensor_copy(st[:], ps_o[:])
                        else:
                            nc.scalar.copy(st[:], ps_o[:])
                        nc.sync.dma_start(
                            out_d.ap()[m * 128:(m + 1) * 128, qsl], st[:])

                # group order: x-projection heads (2,3) of the first two
                # q-chunks run first — the dec heads' rope (DVE) then has a
                # ~4-group window instead of needing to be ready by group 3.
                order = [(0, 2), (0, 3), (1, 2), (1, 3), (0, 0), (0, 1),
                         (1, 0), (1, 1),
                         (2, 2), (2, 3), (2, 0), (2, 1),
                         (3, 2), (3, 3), (3, 0), (3, 1)]
                ctxs = {}
                pending = []
                for gi, (qc, h) in enumerate(order):
                    ctxs.setdefault(qc, {})[h] = emit_group(qc, h)
                    if gi == 1:
                        rope_tiles(swp_p, scr_p, [0, 4], nc.vector)
                    if gi == 2:
                        rope_tiles(swp_p, scr_p, [1, 5], nc.vector)
                    # out-proj is emitted in two halves, the second after
                    # the NEXT group's block, so its 32 matmuls interleave
                    # with live exp traffic instead of starving ACT
                    if pending:
                        q2, d2 = pending.pop(0)
                        emit_outproj(q2, d2, range(8, 16))
                    if len(ctxs[qc]) == 4:
                        d = ctxs.pop(qc)
                        emit_outproj(qc, d, range(0, 8))
                        pending.append((qc, d))
                for q2, d2 in pending:
                    emit_outproj(q2, d2, range(8, 16))
            scr_cm.__exit__(None, None, None)
            swp_cm.__exit__(None, None, None)
            consts_cm.__exit__(None, None, None)

    nc.compile()
    return nc


def _get_program():
    if "nc" not in _prog_cache:
        _prog_cache["nc"] = _build_program()
    return _prog_cache["nc"]


def _host_shards(x, W_comp, W_q_dec, W_k_dec, W_v_dec, W_rope_q, W_rope_k,
                 W_out):
    import ml_dtypes
    bf16 = ml_dtypes.bfloat16

    inv = 1.0 / (10000.0 ** (np.arange(0, RD, 2, dtype=np.float32) / RD))
    ang = np.arange(S, dtype=np.float32)[:, None] * inv[None, :]     # [S, 32]
    cosT = np.cos(ang).T.astype(np.float32)                          # [32, S]
    sinT = np.sin(ang).T.astype(np.float32)
    cos4 = np.ascontiguousarray(np.tile(cosT, (4, 1))).astype(bf16)  # [128,S]
    sin4s = np.ascontiguousarray(
        np.concatenate([-sinT, sinT], axis=0)).astype(bf16)          # [64, S]

    in_maps = []
    for c in range(NC):
        b, hg = divmod(c, 4)
        xTb = np.ascontiguousarray(x[b].T.astype(bf16))
        w_big = np.ascontiguousarray(np.concatenate(
            [W_comp,
             W_rope_q[:, hg * 256:(hg + 1) * 256],
             W_rope_k[:, hg * 256:(hg + 1) * 256]], axis=1).astype(bf16))
        w_qk = np.ascontiguousarray(np.concatenate(
            [W_q_dec[:, hg * 256:(hg + 1) * 256],
             W_k_dec[:, hg * 256:(hg + 1) * 256]], axis=1).astype(bf16))
        w_v = np.ascontiguousarray(np.concatenate(
            [W_v_dec[:, hg * 256:(hg + 1) * 256],
             W_v_dec[:, 1024 + hg * 256:1024 + (hg + 1) * 256]],
            axis=1).astype(bf16))
        w_o = np.ascontiguousarray(np.concatenate(
            [W_out[hg * 256:(hg + 1) * 256, :],
             W_out[1024 + hg * 256:1024 + (hg + 1) * 256, :]],
            axis=0).astype(bf16))
        in_maps.append({
            "xT": xTb, "w_big": w_big, "w_qk": w_qk, "w_v": w_v, "w_o": w_o,
            "cos4": cos4, "sin4s": sin4s,
        })
    return in_maps


def kernel(x, W_comp, W_q_dec, W_k_dec, W_v_dec, W_rope_q, W_rope_k, W_out,
           _trace=False):
    from concourse import bass_utils

    x = np.asarray(x, np.float32)
    args = [np.asarray(a, np.float32)
            for a in (W_comp, W_q_dec, W_k_dec, W_v_dec,
                      W_rope_q, W_rope_k, W_out)]
    in_maps = _host_shards(x, *args)
    nc = _get_program()
    res = bass_utils.run_bass_kernel_spmd(
        nc, in_maps, core_ids=list(range(NC)), trace=_trace)
    out = np.zeros((B, S, D), np.float32)
    for c in range(NC):
        b = c // 4
        out[b] += res.results[c]["out"].astype(np.float32).T
    if _trace:
        kernel.last_exec_ns = res.exec_time_ns
    return out
